# revision 12
# baseline (speedup 1.0000x reference)
"""Trainium2 Bass kernel for CrossAttentionFusion.

Reference computation (B=4, C=256, Cs=256, CI=128, H=W=64, N=M=4096):
    q = Wq @ x + bq; k = Wk @ z + bk; v = Wv @ z + bv
    att = softmax(q^T k, axis=m);  out = gamma * (v @ att^T) + x

Sharding: 8 cores = 4 batches x 2 query-halves (2048 queries each).
Each core holds full K/V for its batch; weights replicated.

Per-core design:
- Energies are computed *transposed* (eT[m, n], m on partitions) so both the
  QK^T and AV matmuls contract along the partition dim - no transposes of
  the attention matrix anywhere.
- Softmax runs without max subtraction (energies ~N(0, 128), far below fp32
  exp overflow range).
- The PE stream is software-pipelined: energy matmuls for chunk-group g+1
  are emitted before the AV matmuls of group g, so the PE never stalls on
  the ACT exp of the current group.
- The softmax denominator: exp chunks are summed with an add-chain split
  across DVE and GPSIMD, then one ones-vector matmul (scaled by 1/gamma)
  reduces over partitions; DVE reciprocal + gpsimd partition_broadcast give
  gamma/sums broadcast over partitions. Normalization, the v-bias, gamma and
  the residual all fold into the output epilogue:
      out = (v_nb^T p) * (gamma/sums) + gamma*bv + x
- The attention path runs in bf16 (PE native rate; DVE 2x mode); the
  residual path (x) stays fp32, which dilutes attention-path rounding ~30x
  since |gamma * att_out| << |x|.
"""
import sys

if "/opt/trn_rl_repo" not in sys.path:
    sys.path.insert(0, "/opt/trn_rl_repo")

import ml_dtypes
import numpy as np

B, C, CS, CI, H, W = 4, 256, 256, 128, 64, 64
N = H * W            # 4096 keys/values per batch
NQ = N // 2          # 2048 queries per core
N_CORES = 8
NT = NQ // 512       # 4 query tiles of 512
MC = N // 128        # 32 m-chunks of 128
NG = MC // 2         # 16 exp groups of 2 m-chunks

BF16 = ml_dtypes.bfloat16
_CACHE = {}


def _build():
    from concourse import bacc, mybir
    from concourse.tile import TileContext

    f32 = mybir.dt.float32
    f32r = mybir.dt.float32r
    bf16 = mybir.dt.bfloat16
    EXP = mybir.ActivationFunctionType.Exp
    ADD = mybir.AluOpType.add
    MULT = mybir.AluOpType.mult

    nc = bacc.Bacc("TRN2", num_devices=N_CORES, debug=False)

    xm = nc.dram_tensor("xm", [C, NQ], f32r, kind="ExternalInput")
    zf = nc.dram_tensor("zf", [CS, N], bf16, kind="ExternalInput")
    wqt = nc.dram_tensor("wqt", [C, CI], f32r, kind="ExternalInput")
    wkt = nc.dram_tensor("wkt", [CS, CI], bf16, kind="ExternalInput")
    wvt = nc.dram_tensor("wvt", [CS, C], bf16, kind="ExternalInput")
    bq = nc.dram_tensor("bq", [CI, 1], f32, kind="ExternalInput")
    bk = nc.dram_tensor("bk", [CI, 1], f32, kind="ExternalInput")
    gbv = nc.dram_tensor("gbv", [C, 1], f32, kind="ExternalInput")
    gcol = nc.dram_tensor("gcol", [128, 1], f32, kind="ExternalInput")
    onesd = nc.dram_tensor("onesd", [CI, 1], bf16, kind="ExternalInput")
    out = nc.dram_tensor("out", [C, NQ], f32, kind="ExternalOutput")

    with TileContext(nc) as tc:
        with tc.tile_pool(name="const", bufs=1) as cpool, \
             tc.tile_pool(name="big", bufs=1) as bpool, \
             tc.tile_pool(name="vtp", bufs=MC) as vpool, \
             tc.tile_pool(name="work", bufs=3) as wpool, \
             tc.tile_pool(name="ps", bufs=2, space="PSUM") as ps:

            # ---- big activations first on the sync DMA queue -------------
            zf_t = [bpool.tile([128, N], bf16, tag=f"zf{i}", name=f"zf{i}")
                    for i in range(2)]
            xm_t = [bpool.tile([128, NQ], f32r, tag=f"xm{i}", name=f"xm{i}")
                    for i in range(2)]
            for h in range(4):
                hs = slice(h * 1024, (h + 1) * 1024)
                for i in range(2):
                    nc.sync.dma_start(zf_t[i][:, hs],
                                      zf.ap()[i * 128:(i + 1) * 128, hs])
            for i in range(2):
                nc.scalar.dma_start(xm_t[i][:], xm.ap()[i * 128:(i + 1) * 128, :])

            # ---- weights / consts on the gpsimd DMA queue ----------------
            wkt_t = [cpool.tile([128, CI], bf16, tag=f"wkt{i}", name=f"wkt{i}")
                     for i in range(2)]
            wqt_t = [cpool.tile([128, CI], f32r, tag=f"wqt{i}", name=f"wqt{i}")
                     for i in range(2)]
            wvt_t = [cpool.tile([128, C], bf16, tag=f"wvt{i}", name=f"wvt{i}")
                     for i in range(2)]
            bq_t = cpool.tile([CI, 1], f32, tag="bq")
            bk_t = cpool.tile([CI, 1], f32, tag="bk")
            gbv_t = [cpool.tile([128, 1], f32, tag=f"gbv{i}", name=f"gbv{i}")
                     for i in range(2)]
            ones_t = cpool.tile([128, 1], bf16, tag="ones")
            gcol_t = cpool.tile([128, 1], f32, tag="gcol")
            for i in range(2):
                nc.gpsimd.dma_start(wkt_t[i][:], wkt.ap()[i * 128:(i + 1) * 128, :])
            nc.gpsimd.dma_start(bk_t[:], bk.ap())
            for i in range(2):
                nc.gpsimd.dma_start(wvt_t[i][:], wvt.ap()[i * 128:(i + 1) * 128, :])
                nc.gpsimd.dma_start(wqt_t[i][:], wqt.ap()[i * 128:(i + 1) * 128, :])
            nc.gpsimd.dma_start(bq_t[:], bq.ap())
            nc.gpsimd.dma_start(ones_t[:], onesd.ap())
            nc.gpsimd.dma_start(gcol_t[:], gcol.ap())
            for i in range(2):
                nc.gpsimd.dma_start(gbv_t[i][:], gbv.ap()[i * 128:(i + 1) * 128, :])

            k_t = bpool.tile([CI, N], bf16, tag="k")
            q_t = bpool.tile([CI, NQ], bf16, tag="q")
            vt_t = [vpool.tile([128, C], bf16, tag="vt", name=f"vt{_}")
                    for _ in range(MC)]

            # Projections, interleaved K/Q/VT so PE never waits on the DVE
            # PSUM->SBUF copies; VT psums rotate across 3 psum tags.
            vt_tags = ["e", "o0", "o1"]

            def emit_k(mt):
                pk = ps.tile([128, 1024], f32, tag="e", name=f"pk{mt}")
                sl = slice(mt * 512, (mt + 1) * 512)
                nc.tensor.matmul(pk[:, 0:512], wkt_t[0][:], zf_t[0][:, sl],
                                 start=True, stop=False)
                nc.tensor.matmul(pk[:, 0:512], wkt_t[1][:], zf_t[1][:, sl],
                                 start=False, stop=True)
                nc.vector.tensor_scalar_add(k_t[:, sl], pk[:, 0:512], bk_t[:])

            def emit_q(qt):
                pq = ps.tile([128, 1024], f32, tag="e", name=f"pq{qt}")
                sl = slice(qt * 512, (qt + 1) * 512)
                nc.tensor.matmul(pq[:, 0:512], wqt_t[0][:], xm_t[0][:, sl],
                                 start=True, stop=False)
                nc.tensor.matmul(pq[:, 0:512], wqt_t[1][:], xm_t[1][:, sl],
                                 start=False, stop=True)
                nc.vector.tensor_scalar_add(q_t[:, sl], pq[:, 0:512], bq_t[:])

            def emit_vt(mc):
                # VT[m, o] = sum_c z[c, m] WvT[c, o]  (bias folded in epilogue)
                pv = ps.tile([128, 512], f32, tag=vt_tags[mc % 3],
                             name=f"pv{mc}")
                sl = slice(mc * 128, (mc + 1) * 128)
                nc.tensor.matmul(pv[:, 0:C], zf_t[0][:, sl], wvt_t[0][:],
                                 start=True, stop=False)
                nc.tensor.matmul(pv[:, 0:C], zf_t[1][:, sl], wvt_t[1][:],
                                 start=False, stop=True)
                nc.vector.tensor_copy(vt_t[mc][:], pv[:, 0:C])

            for mt in range(8):
                emit_k(mt)
                for mc in range(4 * mt, 4 * mt + 4):
                    emit_vt(mc)
            for qt in range(4):
                emit_q(qt)

            # ---- attention -----------------------------------------------
            def emit_et(g, nsl):
                e_ps = ps.tile([128, 1024], f32, tag="e", name=f"e{g}")
                for j in range(2):
                    mc = 2 * g + j
                    nc.tensor.matmul(
                        e_ps[:, j * 512:(j + 1) * 512],
                        k_t[:, mc * 128:(mc + 1) * 128],
                        q_t[:, nsl], start=True, stop=True)
                return e_ps

            def make_tail(nt, out_ps, acc_d, acc_g):
                # Deferred softmax tail + epilogue for query-tile nt; emitted
                # a few groups into the NEXT tile so the PE stream is never
                # blocked behind the DVE/GPSIMD add chain.
                def tail():
                    nsl = slice(nt * 512, (nt + 1) * 512)
                    nc.vector.tensor_add(acc_d[:], acc_d[:], acc_g[:])
                    sums_ps = ps.tile([1, 512], f32, tag="e", name="sums_ps")
                    nc.tensor.matmul(sums_ps[:], ones_t[:], acc_d[:],
                                     start=True, stop=True)
                    sums_sb = wpool.tile([1, 512], f32, tag="sums_sb")
                    nc.vector.tensor_copy(sums_sb[:], sums_ps[:])
                    recip = wpool.tile([1, 512], f32, tag="recip")
                    nc.vector.reciprocal_approx_fast(recip[:], sums_sb[:])
                    rg_sb = wpool.tile([128, 512], f32, tag="rg")
                    nc.gpsimd.partition_broadcast(rg_sb[:], recip[:])
                    # epilogue: out = out_unnorm * gamma/sums + gamma*bv + x
                    for oc in range(2):
                        csl = slice(oc * 128, (oc + 1) * 128)
                        t_sb = wpool.tile([128, 512], f32, tag="t")
                        nc.vector.scalar_tensor_tensor(
                            t_sb[:], out_ps[oc][:], gcol_t[:], rg_sb[:],
                            op0=MULT, op1=MULT)
                        f_sb = wpool.tile([128, 512], f32, tag="f")
                        nc.vector.scalar_tensor_tensor(
                            f_sb[:], t_sb[:], gbv_t[oc][:], xm_t[oc][:, nsl],
                            op0=ADD, op1=ADD)
                        nc.sync.dma_start(out.ap()[csl, nsl], f_sb[:])
                return tail

            pending_tail = None
            for nt in range(NT):
                nsl = slice(nt * 512, (nt + 1) * 512)
                out_ps = [ps.tile([128, 512], f32, tag=f"o{oc}",
                                  name=f"ops{oc}", bufs=2) for oc in range(2)]
                acc_d = wpool.tile([128, 512], bf16, tag="acc_d", bufs=2)
                acc_g = wpool.tile([128, 512], bf16, tag="acc_g", bufs=2)

                e_next = emit_et(0, nsl)
                for g in range(NG):
                    e_cur = e_next
                    p_t = wpool.tile([128, 1024], bf16, tag="p", bufs=12)
                    nc.scalar.activation(p_t[:], e_cur[:], EXP)
                    if g + 1 < NG:
                        e_next = emit_et(g + 1, nsl)
                    for j in range(2):
                        mc = 2 * g + j
                        for oc in range(2):
                            nc.tensor.matmul(
                                out_ps[oc][:],
                                vt_t[mc][:, oc * 128:(oc + 1) * 128],
                                p_t[:, j * 512:(j + 1) * 512],
                                start=(mc == 0), stop=(mc == MC - 1))
                    # softmax-denominator add chain, split DVE / GPSIMD.
                    # GPSIMD only gets early groups so its slow ops finish
                    # well before the tile's merge.
                    on_gps = (g % 2 == 0) and (2 <= g <= 10)
                    eng, acc = (nc.gpsimd, acc_g) if on_gps else \
                               (nc.vector, acc_d)
                    first = (g == 0) if not on_gps else (g == 2)
                    if first:
                        eng.tensor_add(acc[:], p_t[:, 0:512], p_t[:, 512:1024])
                    else:
                        eng.tensor_add(acc[:], acc[:], p_t[:, 0:512])
                        eng.tensor_add(acc[:], acc[:], p_t[:, 512:1024])
                    if g == 3 and pending_tail is not None:
                        pending_tail()
                        pending_tail = None
                pending_tail = make_tail(nt, out_ps, acc_d, acc_g)
            pending_tail()

    nc.compile()
    return nc


def _get_nc():
    if "nc" not in _CACHE:
        _CACHE["nc"] = _build()
    return _CACHE["nc"]


def kernel(x_main, z_p, Wq, bq, Wk, bk, Wv, bv, gamma, _trace=False):
    from concourse import bass_utils

    nc = _get_nc()

    xm_full = np.ascontiguousarray(np.asarray(x_main, np.float32)).reshape(B, C, N)
    zf_full = np.asarray(z_p, np.float32).reshape(B, CS, N).astype(BF16)
    g = float(np.float32(np.asarray(gamma).reshape(-1)[0]))
    common = {
        "wqt": np.ascontiguousarray(np.asarray(Wq, np.float32).T),
        "wkt": np.ascontiguousarray(np.asarray(Wk, np.float32).T.astype(BF16)),
        "wvt": np.ascontiguousarray(np.asarray(Wv, np.float32).T.astype(BF16)),
        "bq": np.asarray(bq, np.float32).reshape(CI, 1),
        "bk": np.asarray(bk, np.float32).reshape(CI, 1),
        "gbv": (np.float32(g) * np.asarray(bv, np.float32)).reshape(C, 1),
        "onesd": np.ones((CI, 1), BF16),
        "gcol": np.full((128, 1), g, np.float32),
    }
    in_maps = []
    for core in range(N_CORES):
        b, half = divmod(core, 2)
        in_maps.append({
            "xm": np.ascontiguousarray(xm_full[b][:, half * NQ:(half + 1) * NQ]),
            "zf": np.ascontiguousarray(zf_full[b]),
            **common,
        })

    res = bass_utils.run_bass_kernel_spmd(
        nc, in_maps, core_ids=list(range(N_CORES)), trace=_trace)

    out = np.empty((B, C, N), np.float32)
    for core in range(N_CORES):
        b, half = divmod(core, 2)
        out[b][:, half * NQ:(half + 1) * NQ] = res.results[core]["out"]
    if _trace:
        _CACHE["last_result"] = res
    return out.reshape(B, C, H, W)


# revision 13
# speedup vs baseline: 1.0746x; 1.0746x over previous
"""Trainium2 Bass kernel for CrossAttentionFusion.

Reference computation (B=4, C=256, Cs=256, CI=128, H=W=64, N=M=4096):
    q = Wq @ x + bq; k = Wk @ z + bk; v = Wv @ z + bv
    att = softmax(q^T k, axis=m);  out = gamma * (v @ att^T) + x

Sharding: 8 cores = 4 batches x 2 query-halves (2048 queries each).
Each core holds full K/V for its batch; weights replicated.

Per-core design:
- Energies are computed *transposed* (eT[m, n], m on partitions) so both the
  QK^T and AV matmuls contract along the partition dim - no transposes of
  the attention matrix anywhere.
- Softmax runs without max subtraction (energies ~N(0, 128), far below fp32
  exp overflow range).
- The PE stream is software-pipelined: energy matmuls for chunk-group g+1
  are emitted before the AV matmuls of group g, so the PE never stalls on
  the ACT exp of the current group.
- The softmax denominator: exp chunks are summed with an add-chain split
  across DVE and GPSIMD, then one ones-vector matmul (scaled by 1/gamma)
  reduces over partitions; DVE reciprocal + gpsimd partition_broadcast give
  gamma/sums broadcast over partitions. Normalization, the v-bias, gamma and
  the residual all fold into the output epilogue:
      out = (v_nb^T p) * (gamma/sums) + gamma*bv + x
- The attention path runs in bf16 (PE native rate; DVE 2x mode); the
  residual path (x) stays fp32, which dilutes attention-path rounding ~30x
  since |gamma * att_out| << |x|.
"""
import sys

if "/opt/trn_rl_repo" not in sys.path:
    sys.path.insert(0, "/opt/trn_rl_repo")

import ml_dtypes
import numpy as np

B, C, CS, CI, H, W = 4, 256, 256, 128, 64, 64
N = H * W            # 4096 keys/values per batch
NQ = N // 2          # 2048 queries per core
N_CORES = 8
NT = NQ // 512       # 4 query tiles of 512
MC = N // 128        # 32 m-chunks of 128
NG = MC // 2         # 16 exp groups of 2 m-chunks

BF16 = ml_dtypes.bfloat16
_CACHE = {}


def _build():
    from concourse import bacc, mybir
    from concourse.tile import TileContext

    f32 = mybir.dt.float32
    f32r = mybir.dt.float32r
    bf16 = mybir.dt.bfloat16
    EXP = mybir.ActivationFunctionType.Exp
    ADD = mybir.AluOpType.add
    MULT = mybir.AluOpType.mult

    nc = bacc.Bacc("TRN2", num_devices=N_CORES, debug=False)

    xm = nc.dram_tensor("xm", [C, NQ], f32r, kind="ExternalInput")
    zf = nc.dram_tensor("zf", [CS, N], bf16, kind="ExternalInput")
    wqt = nc.dram_tensor("wqt", [C, CI], f32r, kind="ExternalInput")
    wkt = nc.dram_tensor("wkt", [CS, CI], bf16, kind="ExternalInput")
    wvt = nc.dram_tensor("wvt", [CS, C], bf16, kind="ExternalInput")
    bq = nc.dram_tensor("bq", [CI, 1], f32, kind="ExternalInput")
    bk = nc.dram_tensor("bk", [CI, 1], f32, kind="ExternalInput")
    gbv = nc.dram_tensor("gbv", [C, 1], f32, kind="ExternalInput")
    gcol = nc.dram_tensor("gcol", [128, 1], f32, kind="ExternalInput")
    onesd = nc.dram_tensor("onesd", [CI, 1], bf16, kind="ExternalInput")
    out = nc.dram_tensor("out", [C, NQ], f32, kind="ExternalOutput")

    with TileContext(nc) as tc:
        with tc.tile_pool(name="const", bufs=1) as cpool, \
             tc.tile_pool(name="big", bufs=1) as bpool, \
             tc.tile_pool(name="vtp", bufs=MC) as vpool, \
             tc.tile_pool(name="work", bufs=3) as wpool, \
             tc.tile_pool(name="ps", bufs=2, space="PSUM") as ps:

            # ---- big activations first on the sync DMA queue -------------
            zf_t = [bpool.tile([128, N], bf16, tag=f"zf{i}", name=f"zf{i}")
                    for i in range(2)]
            xm_t = [bpool.tile([128, NQ], f32r, tag=f"xm{i}", name=f"xm{i}")
                    for i in range(2)]
            for h in range(4):
                hs = slice(h * 1024, (h + 1) * 1024)
                for i in range(2):
                    nc.sync.dma_start(zf_t[i][:, hs],
                                      zf.ap()[i * 128:(i + 1) * 128, hs])
            for i in range(2):
                nc.scalar.dma_start(xm_t[i][:], xm.ap()[i * 128:(i + 1) * 128, :])

            # ---- weights / consts on the gpsimd DMA queue ----------------
            wkt_t = [cpool.tile([128, CI], bf16, tag=f"wkt{i}", name=f"wkt{i}")
                     for i in range(2)]
            wqt_t = [cpool.tile([128, CI], f32r, tag=f"wqt{i}", name=f"wqt{i}")
                     for i in range(2)]
            wvt_t = [cpool.tile([128, C], bf16, tag=f"wvt{i}", name=f"wvt{i}")
                     for i in range(2)]
            bq_t = cpool.tile([CI, 1], f32, tag="bq")
            bk_t = cpool.tile([CI, 1], f32, tag="bk")
            gbv_t = [cpool.tile([128, 1], f32, tag=f"gbv{i}", name=f"gbv{i}")
                     for i in range(2)]
            ones_t = cpool.tile([128, 1], bf16, tag="ones")
            gcol_t = cpool.tile([128, 1], f32, tag="gcol")
            for i in range(2):
                nc.gpsimd.dma_start(wkt_t[i][:], wkt.ap()[i * 128:(i + 1) * 128, :])
            nc.gpsimd.dma_start(bk_t[:], bk.ap())
            for i in range(2):
                nc.gpsimd.dma_start(wvt_t[i][:], wvt.ap()[i * 128:(i + 1) * 128, :])
                nc.gpsimd.dma_start(wqt_t[i][:], wqt.ap()[i * 128:(i + 1) * 128, :])
            nc.gpsimd.dma_start(bq_t[:], bq.ap())
            nc.gpsimd.dma_start(ones_t[:], onesd.ap())
            nc.gpsimd.dma_start(gcol_t[:], gcol.ap())
            for i in range(2):
                nc.gpsimd.dma_start(gbv_t[i][:], gbv.ap()[i * 128:(i + 1) * 128, :])

            k_t = bpool.tile([CI, N], bf16, tag="k")
            q_t = bpool.tile([CI, NQ], bf16, tag="q")
            vt_t = [vpool.tile([128, C], bf16, tag="vt", name=f"vt{_}")
                    for _ in range(MC)]

            # Projections, interleaved K/Q/VT so PE never waits on the DVE
            # PSUM->SBUF copies; VT psums rotate across 3 psum tags.
            vt_tags = ["e", "o0", "o1"]

            def emit_k(mt):
                pk = ps.tile([128, 1024], f32, tag="e", name=f"pk{mt}")
                sl = slice(mt * 512, (mt + 1) * 512)
                nc.tensor.matmul(pk[:, 0:512], wkt_t[0][:], zf_t[0][:, sl],
                                 start=True, stop=False)
                nc.tensor.matmul(pk[:, 0:512], wkt_t[1][:], zf_t[1][:, sl],
                                 start=False, stop=True)
                nc.vector.tensor_scalar_add(k_t[:, sl], pk[:, 0:512], bk_t[:])

            def emit_q(qt):
                pq = ps.tile([128, 1024], f32, tag="e", name=f"pq{qt}")
                sl = slice(qt * 512, (qt + 1) * 512)
                nc.tensor.matmul(pq[:, 0:512], wqt_t[0][:], xm_t[0][:, sl],
                                 start=True, stop=False)
                nc.tensor.matmul(pq[:, 0:512], wqt_t[1][:], xm_t[1][:, sl],
                                 start=False, stop=True)
                nc.vector.tensor_scalar_add(q_t[:, sl], pq[:, 0:512], bq_t[:])

            def emit_vt(mc):
                # VT[m, o] = sum_c z[c, m] WvT[c, o]  (bias folded in epilogue)
                pv = ps.tile([128, 512], f32, tag=vt_tags[mc % 3],
                             name=f"pv{mc}")
                sl = slice(mc * 128, (mc + 1) * 128)
                nc.tensor.matmul(pv[:, 0:C], zf_t[0][:, sl], wvt_t[0][:],
                                 start=True, stop=False)
                nc.tensor.matmul(pv[:, 0:C], zf_t[1][:, sl], wvt_t[1][:],
                                 start=False, stop=True)
                nc.vector.tensor_copy(vt_t[mc][:], pv[:, 0:C])

            for mt in range(8):
                emit_k(mt)
                for mc in range(4 * mt, 4 * mt + 4):
                    emit_vt(mc)
            for qt in range(4):
                emit_q(qt)

            # ---- attention -----------------------------------------------
            def emit_et(g, nsl):
                e_ps = ps.tile([128, 1024], f32, tag="e", name=f"e{g}")
                for j in range(2):
                    mc = 2 * g + j
                    nc.tensor.matmul(
                        e_ps[:, j * 512:(j + 1) * 512],
                        k_t[:, mc * 128:(mc + 1) * 128],
                        q_t[:, nsl], start=True, stop=True)
                return e_ps

            def make_tail(nt, out_ps, acc_d, acc_g):
                # Deferred softmax tail + epilogue for query-tile nt; emitted
                # a few groups into the NEXT tile so the PE stream is never
                # blocked behind the DVE/GPSIMD add chain.
                def tail():
                    nsl = slice(nt * 512, (nt + 1) * 512)
                    nc.vector.tensor_add(acc_d[:], acc_d[:], acc_g[:])
                    sums_ps = ps.tile([1, 512], f32, tag="e", name="sums_ps")
                    nc.tensor.matmul(sums_ps[:], ones_t[:], acc_d[:],
                                     start=True, stop=True)
                    sums_sb = wpool.tile([1, 512], f32, tag="sums_sb")
                    nc.vector.tensor_copy(sums_sb[:], sums_ps[:])
                    recip = wpool.tile([1, 512], f32, tag="recip")
                    nc.vector.reciprocal_approx_fast(recip[:], sums_sb[:])
                    rg_sb = wpool.tile([128, 512], f32, tag="rg")
                    nc.gpsimd.partition_broadcast(rg_sb[:], recip[:])
                    # epilogue: out = out_unnorm * gamma/sums + gamma*bv + x
                    for oc in range(2):
                        csl = slice(oc * 128, (oc + 1) * 128)
                        t_sb = wpool.tile([128, 512], f32, tag="t")
                        nc.vector.scalar_tensor_tensor(
                            t_sb[:], out_ps[oc][:], gcol_t[:], rg_sb[:],
                            op0=MULT, op1=MULT)
                        f_sb = wpool.tile([128, 512], f32, tag="f")
                        nc.vector.scalar_tensor_tensor(
                            f_sb[:], t_sb[:], gbv_t[oc][:], xm_t[oc][:, nsl],
                            op0=ADD, op1=ADD)
                        nc.sync.dma_start(out.ap()[csl, nsl], f_sb[:])
                return tail

            for nt in range(NT):
                nsl = slice(nt * 512, (nt + 1) * 512)
                out_ps = [ps.tile([128, 512], f32, tag=f"o{oc}",
                                  name=f"ops{oc}", bufs=2) for oc in range(2)]
                acc_d = wpool.tile([128, 512], bf16, tag="acc_d", bufs=2)
                acc_g = wpool.tile([128, 512], bf16, tag="acc_g", bufs=2)

                e_next = emit_et(0, nsl)
                for g in range(NG):
                    e_cur = e_next
                    p_t = wpool.tile([128, 1024], bf16, tag="p", bufs=12)
                    nc.scalar.activation(p_t[:], e_cur[:], EXP)
                    if g + 1 < NG:
                        e_next = emit_et(g + 1, nsl)
                    for j in range(2):
                        mc = 2 * g + j
                        for oc in range(2):
                            nc.tensor.matmul(
                                out_ps[oc][:],
                                vt_t[mc][:, oc * 128:(oc + 1) * 128],
                                p_t[:, j * 512:(j + 1) * 512],
                                start=(mc == 0), stop=(mc == MC - 1))
                    # softmax-denominator add chain, split DVE / GPSIMD.
                    # GPSIMD only gets early groups so its slow ops finish
                    # well before the tile's merge.
                    on_gps = (g % 2 == 0) and (2 <= g <= 10)
                    eng, acc = (nc.gpsimd, acc_g) if on_gps else \
                               (nc.vector, acc_d)
                    first = (g == 0) if not on_gps else (g == 2)
                    if first:
                        eng.tensor_add(acc[:], p_t[:, 0:512], p_t[:, 512:1024])
                    else:
                        eng.tensor_add(acc[:], acc[:], p_t[:, 0:512])
                        eng.tensor_add(acc[:], acc[:], p_t[:, 512:1024])
                make_tail(nt, out_ps, acc_d, acc_g)()

    nc.compile()
    return nc


def _get_nc():
    if "nc" not in _CACHE:
        _CACHE["nc"] = _build()
    return _CACHE["nc"]


def kernel(x_main, z_p, Wq, bq, Wk, bk, Wv, bv, gamma, _trace=False):
    from concourse import bass_utils

    nc = _get_nc()

    xm_full = np.ascontiguousarray(np.asarray(x_main, np.float32)).reshape(B, C, N)
    zf_full = np.asarray(z_p, np.float32).reshape(B, CS, N).astype(BF16)
    g = float(np.float32(np.asarray(gamma).reshape(-1)[0]))
    common = {
        "wqt": np.ascontiguousarray(np.asarray(Wq, np.float32).T),
        "wkt": np.ascontiguousarray(np.asarray(Wk, np.float32).T.astype(BF16)),
        "wvt": np.ascontiguousarray(np.asarray(Wv, np.float32).T.astype(BF16)),
        "bq": np.asarray(bq, np.float32).reshape(CI, 1),
        "bk": np.asarray(bk, np.float32).reshape(CI, 1),
        "gbv": (np.float32(g) * np.asarray(bv, np.float32)).reshape(C, 1),
        "onesd": np.ones((CI, 1), BF16),
        "gcol": np.full((128, 1), g, np.float32),
    }
    in_maps = []
    for core in range(N_CORES):
        b, half = divmod(core, 2)
        in_maps.append({
            "xm": np.ascontiguousarray(xm_full[b][:, half * NQ:(half + 1) * NQ]),
            "zf": np.ascontiguousarray(zf_full[b]),
            **common,
        })

    res = bass_utils.run_bass_kernel_spmd(
        nc, in_maps, core_ids=list(range(N_CORES)), trace=_trace)

    out = np.empty((B, C, N), np.float32)
    for core in range(N_CORES):
        b, half = divmod(core, 2)
        out[b][:, half * NQ:(half + 1) * NQ] = res.results[core]["out"]
    if _trace:
        _CACHE["last_result"] = res
    return out.reshape(B, C, H, W)


# revision 14
# speedup vs baseline: 1.0900x; 1.0143x over previous
"""Trainium2 Bass kernel for CrossAttentionFusion.

Reference computation (B=4, C=256, Cs=256, CI=128, H=W=64, N=M=4096):
    q = Wq @ x + bq; k = Wk @ z + bk; v = Wv @ z + bv
    att = softmax(q^T k, axis=m);  out = gamma * (v @ att^T) + x

Sharding: 8 cores = 4 batches x 2 query-halves (2048 queries each).
Each core holds full K/V for its batch; weights replicated.

Per-core design:
- Energies are computed *transposed* (eT[m, n], m on partitions) so both the
  QK^T and AV matmuls contract along the partition dim - no transposes of
  the attention matrix anywhere.
- Softmax runs without max subtraction (energies ~N(0, 128), far below fp32
  exp overflow range).
- The PE stream is software-pipelined: energy matmuls for chunk-group g+1
  are emitted before the AV matmuls of group g, so the PE never stalls on
  the ACT exp of the current group.
- The softmax denominator: exp chunks are summed with an add-chain split
  across DVE and GPSIMD, then one ones-vector matmul (scaled by 1/gamma)
  reduces over partitions; DVE reciprocal + gpsimd partition_broadcast give
  gamma/sums broadcast over partitions. Normalization, the v-bias, gamma and
  the residual all fold into the output epilogue:
      out = (v_nb^T p) * (gamma/sums) + gamma*bv + x
- The attention path runs in bf16 (PE native rate; DVE 2x mode); the
  residual path (x) stays fp32, which dilutes attention-path rounding ~30x
  since |gamma * att_out| << |x|.
"""
import sys

if "/opt/trn_rl_repo" not in sys.path:
    sys.path.insert(0, "/opt/trn_rl_repo")

import ml_dtypes
import numpy as np

B, C, CS, CI, H, W = 4, 256, 256, 128, 64, 64
N = H * W            # 4096 keys/values per batch
NQ = N // 2          # 2048 queries per core
N_CORES = 8
NT = NQ // 512       # 4 query tiles of 512
MC = N // 128        # 32 m-chunks of 128
NG = MC // 2         # 16 exp groups of 2 m-chunks

BF16 = ml_dtypes.bfloat16
_CACHE = {}


def _build():
    from concourse import bacc, mybir
    from concourse.tile import TileContext

    f32 = mybir.dt.float32
    f32r = mybir.dt.float32r
    bf16 = mybir.dt.bfloat16
    EXP = mybir.ActivationFunctionType.Exp
    ADD = mybir.AluOpType.add
    MULT = mybir.AluOpType.mult

    nc = bacc.Bacc("TRN2", num_devices=N_CORES, debug=False)

    xm = nc.dram_tensor("xm", [C, NQ], f32r, kind="ExternalInput")
    xmb = nc.dram_tensor("xmb", [C, NQ], bf16, kind="ExternalInput")
    zf = nc.dram_tensor("zf", [CS, N], bf16, kind="ExternalInput")
    wqt = nc.dram_tensor("wqt", [C, CI], bf16, kind="ExternalInput")
    wkt = nc.dram_tensor("wkt", [CS, CI], bf16, kind="ExternalInput")
    wvt = nc.dram_tensor("wvt", [CS, C], bf16, kind="ExternalInput")
    bq = nc.dram_tensor("bq", [CI, 1], f32, kind="ExternalInput")
    bk = nc.dram_tensor("bk", [CI, 1], f32, kind="ExternalInput")
    gbv = nc.dram_tensor("gbv", [C, 1], f32, kind="ExternalInput")
    gcol = nc.dram_tensor("gcol", [128, 1], f32, kind="ExternalInput")
    onesd = nc.dram_tensor("onesd", [CI, 1], bf16, kind="ExternalInput")
    out = nc.dram_tensor("out", [C, NQ], f32, kind="ExternalOutput")

    with TileContext(nc) as tc:
        with tc.tile_pool(name="const", bufs=1) as cpool, \
             tc.tile_pool(name="big", bufs=1) as bpool, \
             tc.tile_pool(name="vtp", bufs=MC) as vpool, \
             tc.tile_pool(name="work", bufs=3) as wpool, \
             tc.tile_pool(name="ps", bufs=2, space="PSUM") as ps:

            # ---- big activations first on the sync DMA queue -------------
            zf_t = [bpool.tile([128, N], bf16, tag=f"zf{i}", name=f"zf{i}")
                    for i in range(2)]
            xm_t = [bpool.tile([128, NQ], bf16, tag=f"xm{i}", name=f"xm{i}")
                    for i in range(2)]
            for h in range(4):
                hs = slice(h * 1024, (h + 1) * 1024)
                for i in range(2):
                    nc.sync.dma_start(zf_t[i][:, hs],
                                      zf.ap()[i * 128:(i + 1) * 128, hs])
            for i in range(2):
                nc.scalar.dma_start(xm_t[i][:], xmb.ap()[i * 128:(i + 1) * 128, :])

            # ---- weights / consts on the gpsimd DMA queue ----------------
            wkt_t = [cpool.tile([128, CI], bf16, tag=f"wkt{i}", name=f"wkt{i}")
                     for i in range(2)]
            wqt_t = [cpool.tile([128, CI], bf16, tag=f"wqt{i}", name=f"wqt{i}")
                     for i in range(2)]
            wvt_t = [cpool.tile([128, C], bf16, tag=f"wvt{i}", name=f"wvt{i}")
                     for i in range(2)]
            bq_t = cpool.tile([CI, 1], f32, tag="bq")
            bk_t = cpool.tile([CI, 1], f32, tag="bk")
            gbv_t = [cpool.tile([128, 1], f32, tag=f"gbv{i}", name=f"gbv{i}")
                     for i in range(2)]
            ones_t = cpool.tile([128, 1], bf16, tag="ones")
            gcol_t = cpool.tile([128, 1], f32, tag="gcol")
            for i in range(2):
                nc.gpsimd.dma_start(wkt_t[i][:], wkt.ap()[i * 128:(i + 1) * 128, :])
            nc.gpsimd.dma_start(bk_t[:], bk.ap())
            for i in range(2):
                nc.gpsimd.dma_start(wvt_t[i][:], wvt.ap()[i * 128:(i + 1) * 128, :])
                nc.gpsimd.dma_start(wqt_t[i][:], wqt.ap()[i * 128:(i + 1) * 128, :])
            nc.gpsimd.dma_start(bq_t[:], bq.ap())
            nc.gpsimd.dma_start(ones_t[:], onesd.ap())
            nc.gpsimd.dma_start(gcol_t[:], gcol.ap())
            for i in range(2):
                nc.gpsimd.dma_start(gbv_t[i][:], gbv.ap()[i * 128:(i + 1) * 128, :])

            k_t = bpool.tile([CI, N], bf16, tag="k")
            q_t = bpool.tile([CI, NQ], bf16, tag="q")
            vt_t = [vpool.tile([128, C], bf16, tag="vt", name=f"vt{_}")
                    for _ in range(MC)]

            # Projections, interleaved K/Q/VT so PE never waits on the DVE
            # PSUM->SBUF copies; VT psums rotate across 3 psum tags.
            vt_tags = ["e", "o0", "o1"]

            def emit_k(mt):
                pk = ps.tile([128, 1024], f32, tag="e", name=f"pk{mt}")
                sl = slice(mt * 512, (mt + 1) * 512)
                nc.tensor.matmul(pk[:, 0:512], wkt_t[0][:], zf_t[0][:, sl],
                                 start=True, stop=False)
                nc.tensor.matmul(pk[:, 0:512], wkt_t[1][:], zf_t[1][:, sl],
                                 start=False, stop=True)
                nc.vector.tensor_scalar_add(k_t[:, sl], pk[:, 0:512], bk_t[:])

            def emit_q(qt):
                pq = ps.tile([128, 1024], f32, tag="e", name=f"pq{qt}")
                sl = slice(qt * 512, (qt + 1) * 512)
                nc.tensor.matmul(pq[:, 0:512], wqt_t[0][:], xm_t[0][:, sl],
                                 start=True, stop=False)
                nc.tensor.matmul(pq[:, 0:512], wqt_t[1][:], xm_t[1][:, sl],
                                 start=False, stop=True)
                nc.vector.tensor_scalar_add(q_t[:, sl], pq[:, 0:512], bq_t[:])

            def emit_vt(mc):
                # VT[m, o] = sum_c z[c, m] WvT[c, o]  (bias folded in epilogue)
                pv = ps.tile([128, 512], f32, tag=vt_tags[mc % 3],
                             name=f"pv{mc}")
                sl = slice(mc * 128, (mc + 1) * 128)
                nc.tensor.matmul(pv[:, 0:C], zf_t[0][:, sl], wvt_t[0][:],
                                 start=True, stop=False)
                nc.tensor.matmul(pv[:, 0:C], zf_t[1][:, sl], wvt_t[1][:],
                                 start=False, stop=True)
                nc.vector.tensor_copy(vt_t[mc][:], pv[:, 0:C])

            for mt in range(8):
                emit_k(mt)
                for mc in range(4 * mt, 4 * mt + 4):
                    emit_vt(mc)
            for qt in range(4):
                emit_q(qt)

            # ---- attention -----------------------------------------------
            def emit_et(g, nsl):
                e_ps = ps.tile([128, 1024], f32, tag="e", name=f"e{g}")
                for j in range(2):
                    mc = 2 * g + j
                    nc.tensor.matmul(
                        e_ps[:, j * 512:(j + 1) * 512],
                        k_t[:, mc * 128:(mc + 1) * 128],
                        q_t[:, nsl], start=True, stop=True)
                return e_ps

            def make_tail(nt, out_ps, acc_d, acc_g2):
                # Deferred softmax tail + epilogue for query-tile nt; emitted
                # a few groups into the NEXT tile so the PE stream is never
                # blocked behind the DVE/GPSIMD add chain.
                def tail():
                    nsl = slice(nt * 512, (nt + 1) * 512)
                    nc.vector.tensor_add(acc_d[:], acc_d[:], acc_g2[:])
                    sums_ps = ps.tile([1, 512], f32, tag="e", name="sums_ps")
                    nc.tensor.matmul(sums_ps[:], ones_t[:], acc_d[:],
                                     start=True, stop=True)
                    sums_sb = wpool.tile([1, 512], f32, tag="sums_sb")
                    nc.vector.tensor_copy(sums_sb[:], sums_ps[:])
                    recip = wpool.tile([1, 512], f32, tag="recip")
                    nc.vector.reciprocal_approx_fast(recip[:], sums_sb[:])
                    rg_sb = wpool.tile([128, 512], f32, tag="rg")
                    nc.gpsimd.partition_broadcast(rg_sb[:], recip[:])
                    # epilogue: out = out_unnorm * gamma/sums + gamma*bv + x
                    for oc in range(2):
                        csl = slice(oc * 128, (oc + 1) * 128)
                        x_sb = wpool.tile([128, 512], f32r, tag="x", bufs=4,
                                          name=f"x{oc}")
                        nc.sync.dma_start(x_sb[:], xm.ap()[csl, nsl])
                        t_sb = wpool.tile([128, 512], f32, tag="t")
                        nc.vector.scalar_tensor_tensor(
                            t_sb[:], out_ps[oc][:], gcol_t[:], rg_sb[:],
                            op0=MULT, op1=MULT)
                        f_sb = wpool.tile([128, 512], f32, tag="f")
                        nc.vector.scalar_tensor_tensor(
                            f_sb[:], t_sb[:], gbv_t[oc][:], x_sb[:],
                            op0=ADD, op1=ADD)
                        nc.sync.dma_start(out.ap()[csl, nsl], f_sb[:])
                return tail

            for nt in range(NT):
                nsl = slice(nt * 512, (nt + 1) * 512)
                out_ps = [ps.tile([128, 512], f32, tag=f"o{oc}",
                                  name=f"ops{oc}", bufs=2) for oc in range(2)]
                acc_d1 = wpool.tile([128, 512], bf16, tag="acc_d1", bufs=2)
                acc_d2 = wpool.tile([128, 512], bf16, tag="acc_d2", bufs=2)
                acc_g = wpool.tile([128, 512], bf16, tag="acc_g", bufs=2)
                dve_cnt = [0]

                e_next = emit_et(0, nsl)
                for g in range(NG):
                    e_cur = e_next
                    p_t = wpool.tile([128, 1024], bf16, tag="p", bufs=12)
                    nc.scalar.activation(p_t[:], e_cur[:], EXP)
                    if g + 1 < NG:
                        e_next = emit_et(g + 1, nsl)
                    for j in range(2):
                        mc = 2 * g + j
                        for oc in range(2):
                            nc.tensor.matmul(
                                out_ps[oc][:],
                                vt_t[mc][:, oc * 128:(oc + 1) * 128],
                                p_t[:, j * 512:(j + 1) * 512],
                                start=(mc == 0), stop=(mc == MC - 1))
                    # softmax-denominator add chain, split DVE / GPSIMD.
                    # GPSIMD only gets early groups so its slow ops are done
                    # mid-tile; DVE alternates two accumulators to halve the
                    # terminal serial latency.
                    on_gps = (g % 2 == 0) and (2 <= g <= 10)
                    if on_gps:
                        eng, acc, first = nc.gpsimd, acc_g, (g == 2)
                    else:
                        eng = nc.vector
                        acc = acc_d1 if dve_cnt[0] % 2 == 0 else acc_d2
                        first = dve_cnt[0] < 2
                        dve_cnt[0] += 1
                    if first:
                        eng.tensor_add(acc[:], p_t[:, 0:512], p_t[:, 512:1024])
                    else:
                        eng.tensor_add(acc[:], acc[:], p_t[:, 0:512])
                        eng.tensor_add(acc[:], acc[:], p_t[:, 512:1024])
                    if g == 11:
                        nc.vector.tensor_add(acc_d1[:], acc_d1[:], acc_g[:])
                make_tail(nt, out_ps, acc_d1, acc_d2)()

    nc.compile()
    return nc


def _get_nc():
    if "nc" not in _CACHE:
        _CACHE["nc"] = _build()
    return _CACHE["nc"]


def kernel(x_main, z_p, Wq, bq, Wk, bk, Wv, bv, gamma, _trace=False):
    from concourse import bass_utils

    nc = _get_nc()

    xm_full = np.ascontiguousarray(np.asarray(x_main, np.float32)).reshape(B, C, N)
    zf_full = np.asarray(z_p, np.float32).reshape(B, CS, N).astype(BF16)
    g = float(np.float32(np.asarray(gamma).reshape(-1)[0]))
    common = {
        "wqt": np.ascontiguousarray(np.asarray(Wq, np.float32).T.astype(BF16)),
        "wkt": np.ascontiguousarray(np.asarray(Wk, np.float32).T.astype(BF16)),
        "wvt": np.ascontiguousarray(np.asarray(Wv, np.float32).T.astype(BF16)),
        "bq": np.asarray(bq, np.float32).reshape(CI, 1),
        "bk": np.asarray(bk, np.float32).reshape(CI, 1),
        "gbv": (np.float32(g) * np.asarray(bv, np.float32)).reshape(C, 1),
        "onesd": np.ones((CI, 1), BF16),
        "gcol": np.full((128, 1), g, np.float32),
    }
    in_maps = []
    for core in range(N_CORES):
        b, half = divmod(core, 2)
        in_maps.append({
            "xm": np.ascontiguousarray(xm_full[b][:, half * NQ:(half + 1) * NQ]),
            "xmb": np.ascontiguousarray(
                xm_full[b][:, half * NQ:(half + 1) * NQ].astype(BF16)),
            "zf": np.ascontiguousarray(zf_full[b]),
            **common,
        })

    res = bass_utils.run_bass_kernel_spmd(
        nc, in_maps, core_ids=list(range(N_CORES)), trace=_trace)

    out = np.empty((B, C, N), np.float32)
    for core in range(N_CORES):
        b, half = divmod(core, 2)
        out[b][:, half * NQ:(half + 1) * NQ] = res.results[core]["out"]
    if _trace:
        _CACHE["last_result"] = res
    return out.reshape(B, C, H, W)
